# revision 1
# baseline (speedup 1.0000x reference)
"""EvolveGCN (2-layer) Trainium2 Bass kernel, 8-way sharded.

Key algebraic reduction: the mat-GRU that evolves the GCN weights depends only
on the previous weights (never on data), and layer outputs at time t depend
only on inputs at time t.  Since the model returns h2[T-1] only, the entire
computation collapses to:

    W1* = matGRU^4(W1);  W2* = matGRU^4(W2)        (tiny 128x128 host math)
    h1  = rrelu(A3 @ (X3 @ W1*))
    out = rrelu(A3 @ (h1 @ W2*))

Sharding: output rows (nodes) are range-partitioned across the 8 cores.  Each
core builds its [6250,128] slice of the dense table (X@W / h1@W), an AllGather
replicates the fp16 table to every core, and per-core SWDGE dma_gather pulls
the per-edge messages.  The sparse scatter (segment-sum by row) is done as a
sequence of one-hot-times-val fp16 matmuls on the tensor engine: edges are
host-sorted into 32-row windows, padded to 128-edge chunks; each chunk's S
block [128 edges, 32 rows] carries val at (edge, row) so PSUM accumulates the
weighted segment sums directly.  dma_gather indices are int16, so edges are
split into two groups (table row < 32768 / >= 32768) gathered with different
table base offsets.  Both layers share the same adjacency (t=3), so the idx/S
structures are built once and used twice.
"""

import sys
import numpy as np

for _p in ("/opt/trn_rl_repo",):
    if _p not in sys.path:
        sys.path.insert(0, _p)

T, N, E, F = 4, 50000, 800000, 128
NC = 8
NPC = N // NC            # 6250 nodes per core
RTP = 6272               # padded rows per core (49 tiles of 128)
NT = RTP // 128          # 49 row tiles per core
WROWS = 64               # scatter window rows (matmul M)
NW = RTP // WROWS        # 196 windows per core
SPLIT = 32768            # int16 index limit
SLOPE = 11.0 / 48.0      # torch RReLU eval negative slope
SEGP = 2                 # row tiles per gather segment

# debug bisection flags
DBG_NO_GATHER = False
DBG_NO_SPMM_MM = False
DBG_ONE_LAYER = False
REPS = 1  # timing: chain the whole pipeline N times
PHASE = "all"  # all | table | gather | mm  (timing bisection)
SIM1 = False  # single-core, no-collective variant for TimelineSim


def _evolve(W0, gW, gU, gb, steps=T):
    def sig(x):
        return 1.0 / (1.0 + np.exp(-x))

    Q = W0.astype(np.float64)
    gW = gW.astype(np.float64)
    gU = gU.astype(np.float64)
    gb = gb.astype(np.float64)
    for _ in range(steps):
        z = sig(gW[0] @ Q + gU[0] @ Q + gb[0])
        r = sig(gW[1] @ Q + gU[1] @ Q + gb[1])
        h = np.tanh(gW[2] @ Q + gU[2] @ (r * Q) + gb[2])
        Q = (1.0 - z) * Q + z * h
    return Q.astype(np.float32)


def _prep_edges(row, col, val):
    """Host-side edge schedule. Returns per-core input arrays + shared chunk
    schedule (identical across cores, baked into the single SPMD program)."""
    tcol = (col // NPC) * RTP + (col % NPC)     # remapped table row
    corei = row // NPC
    rl = row % NPC
    win = rl // WROWS
    rr = rl % WROWS
    grp = (tcol >= SPLIT).astype(np.int64)

    # counts[core, grp, win]
    counts = np.zeros((NC, 2, NW), np.int64)
    np.add.at(counts, (corei, grp, win), 1)
    # chunks per (grp, win): max over cores so one schedule fits all
    CC = -(-counts // 128)      # ceildiv
    CC = CC.max(axis=0)         # [2, NW]
    # ensure every window has >= 1 chunk so its PSUM rows get written
    empty = (CC[0] + CC[1]) == 0
    CC[0][empty] = 1

    baseA = np.zeros(NW + 1, np.int64)
    baseA[1:] = np.cumsum(CC[0])
    baseB = np.zeros(NW + 1, np.int64)
    baseB[1:] = np.cumsum(CC[1])
    NCHA, NCHB = int(baseA[-1]), int(baseB[-1])
    NA, NB = NCHA * 128, NCHB * 128

    idxa = np.zeros((NC, 128, NA // 16), np.int16)
    idxb = np.zeros((NC, 128, NB // 16), np.int16)
    sa = np.zeros((NC, 128, NCHA * WROWS), np.float16)
    sb = np.zeros((NC, 128, NCHB * WROWS), np.float16)

    for i in range(NC):
        for g, (base, idxg, sg) in enumerate(
            ((baseA, idxa, sa), (baseB, idxb, sb))
        ):
            m = (corei == i) & (grp == g)
            ew, err = win[m], rr[m]
            etc = tcol[m] - g * SPLIT
            ev = val[m]
            # stable order by window; slot within window = running position
            order = np.argsort(ew, kind="stable")
            ew, err, etc, ev = ew[order], err[order], etc[order], ev[order]
            # slot index within each window
            winstart = np.searchsorted(ew, np.arange(NW))
            pos = np.arange(ew.size) - winstart[ew]
            slot = base[ew] * 128 + pos
            assert (pos < (base[ew + 1] - base[ew]) * 128).all()
            # gather idx array: edge e at [e%16, e//16]
            flat = np.zeros(base[-1] * 128, np.int16)
            flat[slot] = etc.astype(np.int16)
            idxg[i][:16] = flat.reshape(-1, 16).T
            idxg[i] = np.tile(idxg[i][:16], (8, 1))
            # S: [partition = slot%128, (chunk = slot//128)*WROWS + rr] = val
            sflat = sg[i].reshape(-1)
            sidx = (slot % 128) * (base[-1] * WROWS) + (slot // 128) * WROWS + err
            sflat[sidx] = ev.astype(np.float16)

    return CC, baseA, baseB, idxa, idxb, sa, sb


def _build_program(CC, baseA, baseB, NCHA, NCHB):
    import concourse.bass as bass
    import concourse.tile as tile
    from concourse import bacc, mybir
    from concourse.masks import make_identity
    from contextlib import ExitStack

    F32, F16, I16 = mybir.dt.float32, mybir.dt.float16, mybir.dt.int16
    NA, NB = NCHA * 128, NCHB * 128

    nc = bacc.Bacc(
        "TRN2", target_bir_lowering=False, debug=False,
        num_devices=(1 if SIM1 else NC),
    )
    xs_d = nc.dram_tensor("xs", [RTP, F], F32, kind="ExternalInput")
    w1_d = nc.dram_tensor("w1", [F, F], F32, kind="ExternalInput")
    w2_d = nc.dram_tensor("w2", [F, F], F32, kind="ExternalInput")
    idxa_d = nc.dram_tensor("idxa", [128, NA // 16], I16, kind="ExternalInput")
    idxb_d = nc.dram_tensor("idxb", [128, NB // 16], I16, kind="ExternalInput")
    sa_d = nc.dram_tensor("sa", [128, NCHA * WROWS], F16, kind="ExternalInput")
    sb_d = nc.dram_tensor("sb", [128, NCHB * WROWS], F16, kind="ExternalInput")
    out_d = nc.dram_tensor("out", [RTP, F], F32, kind="ExternalOutput")

    # gather segments: SEGP row tiles each
    segs = []
    for p0 in range(0, NT, SEGP):
        p1 = min(p0 + SEGP, NT)
        w0, w1 = p0 * (128 // WROWS), p1 * (128 // WROWS)
        segs.append((p0, p1, w0, w1))
    max_cha = max(int(baseA[w1] - baseA[w0]) for _, _, w0, w1 in segs)
    max_chb = max(int(baseB[w1] - baseB[w0]) for _, _, w0, w1 in segs)

    with tile.TileContext(nc) as tc, ExitStack() as ctx:
        const = ctx.enter_context(tc.tile_pool(name="const", bufs=1))
        xin = ctx.enter_context(tc.tile_pool(name="xin", bufs=8))
        tps = ctx.enter_context(tc.tile_pool(name="tps", bufs=2, space="PSUM"))
        xtp = ctx.enter_context(tc.tile_pool(name="xtp", bufs=2))
        tsh = ctx.enter_context(tc.tile_pool(name="tsh", bufs=8))
        accp = ctx.enter_context(tc.tile_pool(name="accp", bufs=6, space="PSUM"))
        msgp = ctx.enter_context(tc.tile_pool(name="msgp", bufs=3))
        rrp = ctx.enter_context(tc.tile_pool(name="rrp", bufs=4))
        big = ctx.enter_context(tc.tile_pool(name="big", bufs=1))
        dram = ctx.enter_context(tc.tile_pool(name="dram", bufs=1, space="DRAM"))

        ident = const.tile([128, 128], F32)
        make_identity(nc, ident[:])
        w1_sb = const.tile([F, F], F32)
        nc.sync.dma_start(w1_sb[:], w1_d[:, :])
        w2_sb = const.tile([F, F], F32)
        nc.sync.dma_start(w2_sb[:], w2_d[:, :])
        idxa_sb = big.tile([128, NA // 16], I16)
        nc.sync.dma_start(idxa_sb[:], idxa_d[:, :])
        idxb_sb = big.tile([128, NB // 16], I16)
        nc.sync.dma_start(idxb_sb[:], idxb_d[:, :])
        sa_sb = big.tile([128, NCHA * WROWS], F16)
        nc.sync.dma_start(sa_sb[:], sa_d[:, :])
        sb_sb = big.tile([128, NCHB * WROWS], F16)
        nc.sync.dma_start(sb_sb[:], sb_d[:, :])
        h1_sb = big.tile([128, NT * 128], F32)

        def build_table(w_sb, shard, table, from_dram):
            for t in range(NT):
                if from_dram:
                    xt_in = xin.tile([128, 128], F32, tag="xin")
                    nc.sync.dma_start(xt_in[:], xs_d[t * 128 : (t + 1) * 128, :])
                    src = xt_in[:]
                else:
                    src = h1_sb[:, t * 128 : (t + 1) * 128]
                tp = tps.tile([128, 128], F32, tag="tp")
                nc.tensor.transpose(tp[:], src, ident[:])
                xts = xtp.tile([128, 128], F32, tag="xts")
                nc.vector.tensor_copy(xts[:], tp[:])
                mp = tps.tile([128, 128], F32, tag="tp")
                nc.tensor.matmul(
                    out=mp[:], lhsT=xts[:], rhs=w_sb[:], start=True, stop=True
                )
                sh = tsh.tile([128, 128], F16, tag="sh")
                nc.scalar.activation(
                    sh[:], mp[:], mybir.ActivationFunctionType.Copy
                )
                nc.sync.dma_start(shard[t * 128 : (t + 1) * 128, :], sh[:])
            if SIM1:
                for r in range(NC):
                    nc.sync.dma_start(table[r * RTP : (r + 1) * RTP, :], shard[:])
            else:
                nc.gpsimd.collective_compute(
                    "AllGather",
                    mybir.AluOpType.bypass,
                    replica_groups=[list(range(NC))],
                    ins=[shard.opt()],
                    outs=[table.opt()],
                )

        def spmm(table, emit):
            for si, (p0, p1, w0, w1) in enumerate(segs):
                ca0, ca1 = int(baseA[w0]), int(baseA[w1])
                cb0, cb1 = int(baseB[w0]), int(baseB[w1])
                na, nb = (ca1 - ca0) * 128, (cb1 - cb0) * 128
                msga = msgp.tile([128, max_cha, 128], F16, tag="msga")
                msgb = msgp.tile([128, max_chb, 128], F16, tag="msgb")
                if DBG_NO_GATHER or PHASE == "mm":
                    nc.vector.memset(msga[:], 0)
                    nc.vector.memset(msgb[:], 0)
                skip_g = DBG_NO_GATHER or PHASE == "mm"
                if na and not skip_g:
                    nc.gpsimd.dma_gather(
                        out_ap=msga[:, : ca1 - ca0, :],
                        in_ap=table[:SPLIT, :],
                        idxs_ap=idxa_sb[:, ca0 * 8 : ca1 * 8],
                        num_idxs=na,
                        num_idxs_reg=na,
                        elem_size=F,
                        single_packet=False,
                    )
                if nb and not skip_g:
                    nc.gpsimd.dma_gather(
                        out_ap=msgb[:, : cb1 - cb0, :],
                        in_ap=table[SPLIT:, :],
                        idxs_ap=idxb_sb[:, cb0 * 8 : cb1 * 8],
                        num_idxs=nb,
                        num_idxs_reg=nb,
                        elem_size=F,
                        single_packet=False,
                    )
                if PHASE == "gather":
                    continue
                WQ = 128 // WROWS
                for pt in range(p0, p1):
                    acc = accp.tile([128, 128], mybir.dt.float32, tag="acc")
                    if DBG_NO_SPMM_MM:
                        nc.vector.tensor_scalar_mul(acc[:], msga[:, 0, :], 0.0)
                        emit(pt, acc)
                        continue
                    for q in range(WQ):
                        w = pt * WQ + q
                        nw_ch = int(
                            baseA[w + 1] - baseA[w] + baseB[w + 1] - baseB[w]
                        )
                        k = 0
                        for gc in range(int(baseA[w]), int(baseA[w + 1])):
                            nc.tensor.matmul(
                                out=acc[WROWS * q : WROWS * (q + 1), :],
                                lhsT=sa_sb[:, gc * WROWS : (gc + 1) * WROWS],
                                rhs=msga[:, gc - ca0, :],
                                start=(k == 0),
                                stop=(k == nw_ch - 1),
                                tile_position=(0, WROWS * q),
                            )
                            k += 1
                        for gc in range(int(baseB[w]), int(baseB[w + 1])):
                            nc.tensor.matmul(
                                out=acc[WROWS * q : WROWS * (q + 1), :],
                                lhsT=sb_sb[:, gc * WROWS : (gc + 1) * WROWS],
                                rhs=msgb[:, gc - cb0, :],
                                start=(k == 0),
                                stop=(k == nw_ch - 1),
                                tile_position=(0, WROWS * q),
                            )
                            k += 1
                    emit(pt, acc)

        if PHASE in ("gather", "mm"):
            # tables built once; spmm phase repeated
            shard1 = dram.tile([RTP, F], F16)
            table1 = dram.tile([NC * RTP, F], F16, addr_space="Shared")
            build_table(w1_sb, shard1, table1, from_dram=True)

            def emitp(pt, acc):
                tmp = rrp.tile([128, 128], F32, tag="rtmp")
                nc.vector.tensor_scalar_mul(tmp[:], acc[:], SLOPE)
                res = rrp.tile([128, 128], F32, tag="res")
                nc.vector.tensor_tensor(
                    out=res[:], in0=tmp[:], in1=acc[:], op=mybir.AluOpType.max
                )
                nc.sync.dma_start(out_d[pt * 128 : (pt + 1) * 128, :], res[:])

            for _rep in range(REPS):
                spmm(table1, emitp)

        # ---- layer 1
        for _rep in range(REPS if PHASE not in ("gather", "mm") else 0):
            shard1 = dram.tile([RTP, F], F16, name=f"shard1_{_rep}")
            shard2 = dram.tile([RTP, F], F16, name=f"shard2_{_rep}")
            _aspace = "Local" if SIM1 else "Shared"
            table1 = dram.tile([NC * RTP, F], F16, addr_space=_aspace, name=f"table1_{_rep}")
            table2 = dram.tile([NC * RTP, F], F16, addr_space=_aspace, name=f"table2_{_rep}")
            build_table(w1_sb, shard1, table1, from_dram=True)
            if PHASE == "table":
                continue

            def emit1(pt, acc):
                tmp = rrp.tile([128, 128], F32, tag="rtmp")
                nc.vector.tensor_scalar_mul(tmp[:], acc[:], SLOPE)
                nc.vector.tensor_tensor(
                    out=h1_sb[:, pt * 128 : (pt + 1) * 128],
                    in0=tmp[:],
                    in1=acc[:],
                    op=mybir.AluOpType.max,
                )

            spmm(table1, emit1)

            # ---- layer 2
            build_table(w2_sb, shard2, table2, from_dram=False)

            def emit2(pt, acc):
                tmp = rrp.tile([128, 128], F32, tag="rtmp")
                nc.vector.tensor_scalar_mul(tmp[:], acc[:], SLOPE)
                res = rrp.tile([128, 128], F32, tag="res")
                nc.vector.tensor_tensor(
                    out=res[:], in0=tmp[:], in1=acc[:], op=mybir.AluOpType.max
                )
                nc.sync.dma_start(out_d[pt * 128 : (pt + 1) * 128, :], res[:])

            spmm(table2, emit2)

    nc.compile()
    return nc


def kernel(
    features,
    adj_row,
    adj_col,
    adj_val,
    W1,
    g1_W,
    g1_U,
    g1_b,
    W2,
    g2_W,
    g2_U,
    g2_b,
    _run_kwargs=None,
):
    from concourse.bass_utils import run_bass_kernel_spmd

    X = np.asarray(features[T - 1], dtype=np.float32)
    row = np.asarray(adj_row[T - 1], dtype=np.int64)
    col = np.asarray(adj_col[T - 1], dtype=np.int64)
    val = np.asarray(adj_val[T - 1], dtype=np.float32)

    W1f = _evolve(np.asarray(W1), np.asarray(g1_W), np.asarray(g1_U), np.asarray(g1_b))
    W2f = _evolve(np.asarray(W2), np.asarray(g2_W), np.asarray(g2_U), np.asarray(g2_b))

    CC, baseA, baseB, idxa, idxb, sa, sb = _prep_edges(row, col, val)
    NCHA, NCHB = int(baseA[-1]), int(baseB[-1])

    nc = _build_program(CC, baseA, baseB, NCHA, NCHB)

    xs_pad = np.zeros((NC, RTP, F), np.float32)
    xs_pad[:, :NPC] = X.reshape(NC, NPC, F)

    in_maps = [
        {
            "xs": xs_pad[i],
            "w1": W1f,
            "w2": W2f,
            "idxa": idxa[i],
            "idxb": idxb[i],
            "sa": sa[i],
            "sb": sb[i],
        }
        for i in range(NC)
    ]
    res = run_bass_kernel_spmd(
        nc, in_maps, core_ids=list(range(NC)), **(_run_kwargs or {})
    )
    out = np.concatenate([res.results[i]["out"][:NPC] for i in range(NC)], axis=0)
    if _run_kwargs:
        kernel.last_results = res
    return out

